# revision 20
# baseline (speedup 1.0000x reference)
"""Trainium2 Bass kernel for nn_MixquantLinear: O = ((dequant4(V) * S) @ dequant4(U)).T.

Output O is [4096, 4096] fp32, built purely from the GPTQ-quantized weights
(the activation input `x` is dead code in the reference). Sharding: 4 slices
over output rows (o) x 2 over output cols (i) -> 8 cores, no collectives;
host concatenates the blocks.

Key ideas:
  - The contraction index r is relabeled r' = j*128 + p with r = 8p + j
    (word p, nibble j). A full-width shift+mask of the packed U words then
    directly yields k-tile j of the lhsT operand in [r, o] layout -- no PE
    transposes. V and all tables are permuted to match.
  - Packed words are viewed as uint16 halves, host-deinterleaved so every
    device unpack instruction reads AND writes contiguously (the resulting
    within-128-block i permutation is undone on the host at assembly).
  - U scale/zero tables are partition-broadcast with one K=128 indicator
    matmul; U dequant is two wide tensor_tensor ops per k-tile.
  - V dequant: per-group fused affine q*a + b (a = sv*S, b = -(zv+1)*a),
    split across ACT/DVE.
  - fp16 matmuls (k=128 tiles, N=512) accumulate fp32 in PSUM; wave A is
    k-layered to chase strip-0 dequant; output flushed as fp16.
Host-side work is layout-only (slicing/permuting packed words and tables,
column reordering and fp16->fp32 cast of the output).
"""

import numpy as np

import concourse.bass as bass
import concourse.mybir as mybir
import concourse.tile as tile
from concourse import bacc
from concourse.bass_utils import run_bass_kernel_spmd

IN_SIZE = 4096
OUT_SIZE = 4096
RANK = 1024
PACK = 8
P_O = 4
P_I = 2
O_SL = OUT_SIZE // P_O    # 1024
I_SL = IN_SIZE // P_I     # 2048
N_CORES = P_O * P_I
J = RANK // 128           # 8 k-tiles
N_STRIPS = 2
STRIP = I_SL // N_STRIPS  # 1024
NG = 16                   # V groups per core slice (I_SL / 128)

F16 = mybir.dt.float16
F32 = mybir.dt.float32
I32 = mybir.dt.int32
U16 = mybir.dt.uint16
Alu = mybir.AluOpType
Act = mybir.ActivationFunctionType

_NC_CACHE = None
TRACE = False
LAST_RESULTS = None


def _build_nc():
    nc = bacc.Bacc("TRN2", target_bir_lowering=False)

    qut = nc.dram_tensor("qut", [128, O_SL], I32, kind="ExternalInput")
    qvt = nc.dram_tensor("qvt", [128, N_STRIPS * J * 128], I32, kind="ExternalInput")
    dma_u = nc.dram_tensor("dma_u", [8, 1152], I32, kind="ExternalInput")
    dma_v = nc.dram_tensor("dma_v", [128, 344], I32, kind="ExternalInput")
    out = nc.dram_tensor("out", [O_SL, I_SL], F16, kind="ExternalOutput")

    with tile.TileContext(nc) as tc:
        with (
            tc.tile_pool(name="const", bufs=1) as cp,
            tc.tile_pool(name="outsb", bufs=4) as outp,
        ):
            qut_sb = cp.tile([128, O_SL], I32, tag="qut")
            qvt_sb = cp.tile([128, N_STRIPS * J * 128], I32, tag="qvt")
            dmu_sb = cp.tile([8, 1152], I32, tag="dmu")
            dmv_sb = cp.tile([128, 344], I32, tag="dmv")
            rhs = [cp.tile([128, J * STRIP], F16, tag=f"rhs{s}", name=f"rhs{s}")
                   for s in range(N_STRIPS)]
            nibv = [cp.tile([128, J * STRIP], U16, tag=f"nv{s}", name=f"nv{s}")
                    for s in range(N_STRIPS)]
            nibu = cp.tile([128, J * O_SL], U16, tag="nua")
            lhsT = cp.tile([128, J * O_SL], F16, tag="lhsT")
            zu8 = cp.tile([8, O_SL], I32, tag="zu8")
            rhs_bc = cp.tile([128, 2 * O_SL], F16, tag="rhsbc")
            zub_b = cp.tile([128, O_SL], F16, tag="zubb")
            su_b = cp.tile([128, O_SL], F16, tag="sub")
            zv_u = cp.tile([128, J * NG], I32, tag="zvu")
            av = cp.tile([128, J * NG], F32, tag="av")
            bvn = cp.tile([128, J * NG], F32, tag="bvn")
            tneg = cp.tile([128, J * NG], F32, tag="tneg")
            one8 = cp.tile([8, 1], F32, tag="one8")

            qzu_sb = dmu_sb[:, 0:128]
            su8 = dmu_sb[:, 128:1152].bitcast(F32)        # [8, 1024]
            e_sb = dmv_sb[:, 0:64].bitcast(F16)           # [128, 128]
            qzv_t = dmv_sb[:, 64:80]                      # [128, 16]
            svt_p = dmv_sb[:, 80:208].bitcast(F32)        # [128, 128]
            s_p = dmv_sb[:, 208:216].bitcast(F32)         # [128, 8]
            s_exp = dmv_sb[:, 216:344].bitcast(F32)       # [128, 128] S[8p+j] per col
            qvt16 = qvt_sb[:].bitcast(U16)                # [128, 8192]
            qut16 = qut_sb[:].bitcast(U16)                # [128, 2048]: h-major

            # ---- input DMAs: two HWDGE rings in parallel ----
            nc.scalar.dma_start(out=qvt_sb[:, 0:1024], in_=qvt[:, 0:1024])
            nc.sync.dma_start(out=dmv_sb[:], in_=dma_v[:])
            nc.sync.dma_start(out=dmu_sb[:], in_=dma_u[:])
            nc.sync.dma_start(out=qut_sb[:], in_=qut[:])
            nc.sync.dma_start(out=qvt_sb[:, 1024:2048], in_=qvt[:, 1024:2048])

            # ---- PE warmup: ramp the tensor-engine clock during the head ----
            warm = cp.tile([128, 256], F16, tag="warm")
            nc.vector.memset(warm[:], 0.0)
            with tc.tile_pool(name="wps", bufs=2, space="PSUM") as wps:
                wts = [wps.tile([128, 256], F32, tag="wp", name="wp")
                       for _ in range(2)]
                for w in range(44):
                    nc.tensor.matmul(wts[w % 2][:], warm[:, 0:128], warm[:],
                                     start=True, stop=True)

            # rhs_bc rows >=8 must be 0 so the indicator matmul is exact
            # (whole-tile memzero; rows 0..7 are overwritten below)
            nc.scalar.memzero(rhs_bc[:])

            # ---- U zero/scale tables on 8 partitions ----
            zu8_r = zu8[:].rearrange("p (w q) -> p w q", q=PACK)
            for jo in range(PACK):
                nc.vector.tensor_scalar(
                    out=zu8_r[:, :, jo], in0=qzu_sb, scalar1=4 * jo, scalar2=15,
                    op0=Alu.logical_shift_right, op1=Alu.bitwise_and)
            nc.vector.tensor_scalar(
                out=rhs_bc[0:8, 0:O_SL], in0=zu8[:], scalar1=1.0, scalar2=1.0,
                op0=Alu.mult, op1=Alu.add)
            nc.scalar.copy(rhs_bc[0:8, O_SL:2 * O_SL], su8)

            # ---- V tables: zeros unpack, a = sv*S, b = -(zv+1)*a ----
            for j in range(J):
                nc.vector.tensor_scalar(
                    out=zv_u[:, j * NG:(j + 1) * NG], in0=qzv_t, scalar1=4 * j,
                    scalar2=15, op0=Alu.logical_shift_right, op1=Alu.bitwise_and)
            nc.vector.tensor_tensor(av[:], svt_p, s_exp, Alu.mult)
            nc.vector.tensor_scalar(
                out=tneg[:], in0=zv_u[:], scalar1=-1.0, scalar2=-1.0,
                op0=Alu.mult, op1=Alu.add)
            nc.vector.tensor_tensor(bvn[:], tneg[:], av[:], Alu.mult)

            # ---- broadcast (zu+1)|su from 8 partitions to 128 via PE; the
            # psum->sbuf copies sit on DVE right before their consumers ----
            bc_pool_ctx = tc.tile_pool(name="bc", bufs=4, space="PSUM")
            bps = bc_pool_ctx.__enter__()
            bc_pts = []
            for q in range(4):
                pt = bps.tile([128, 512], F32, tag="bc", name="bc")
                nc.tensor.matmul(pt[:], e_sb, rhs_bc[:, q * 512:(q + 1) * 512],
                                 start=True, stop=True)
                bc_pts.append(pt)

            # ---- V unpack: contiguous uint16 shifts (host-deinterleaved) ----
            def v_unpack(s, jlo, jn):
                # in : uint16 idx = s*2048 + j*256 + g*32 + c   (c = 2*iwl + h)
                # out: nib  idx = j*1024 + g*128 + q*32 + c
                srcu = qvt16[:, s * 2048 + jlo * 256:
                             s * 2048 + (jlo + jn) * 256].rearrange(
                    "p (j g c) -> p j g c", j=jn, c=32)
                dstv = nibv[s][:, jlo * STRIP:(jlo + jn) * STRIP].rearrange(
                    "p (j g q c) -> p j g q c", j=jn, q=4, c=32)
                for q in range(4):
                    nc.vector.tensor_scalar(
                        out=dstv[:, :, :, q, :], in0=srcu, scalar1=4 * q, scalar2=15,
                        op0=Alu.logical_shift_right, op1=Alu.bitwise_and)

            # ---- U unpack: contiguous uint16 shifts -> all 8 k-tiles ----
            def u_unpack(q):
                # in : uint16 idx = h*1024 + o ; out: (4h+q)*1024 + o
                srcu = qut16[:].rearrange("p (h o) -> p h o", h=2)
                dstu = nibu[:].rearrange("p (h r) -> p h r", h=2)[
                    :, :, q * O_SL:(q + 1) * O_SL]
                nc.vector.tensor_scalar(
                    out=dstu, in0=srcu, scalar1=4 * q, scalar2=15,
                    op0=Alu.logical_shift_right, op1=Alu.bitwise_and)

            def u_dequant(j):
                lj = lhsT[:, j * O_SL:(j + 1) * O_SL]
                nc.vector.tensor_tensor(
                    lj, nibu[:, j * O_SL:(j + 1) * O_SL], zub_b[:], Alu.subtract)
                nc.vector.tensor_tensor(lj, lj, su_b[:], Alu.mult)

            def v_affine(s, jlo, jn, n_act=4, glo=0, gn=8):
                for j in range(jlo, jlo + jn):
                    for g in range(glo, glo + gn):
                        col = j * NG + s * 8 + g
                        o_ap = rhs[s][:, j * STRIP + g * 128:j * STRIP + (g + 1) * 128]
                        i_ap = nibv[s][:, j * STRIP + g * 128:j * STRIP + (g + 1) * 128]
                        if g < n_act:
                            nc.scalar.activation(
                                o_ap, i_ap, Act.Identity,
                                bias=bvn[:, col:col + 1], scale=av[:, col:col + 1])
                        else:
                            nc.vector.tensor_scalar(
                                out=o_ap, in0=i_ap, scalar1=av[:, col:col + 1],
                                scalar2=bvn[:, col:col + 1], op0=Alu.mult, op1=Alu.add)

            # strip-0: per-j chase (u_unpack(q) yields tiles q and q+4)
            v_unpack(0, 0, 1)
            u_unpack(0)
            for q in range(2):
                nc.vector.tensor_copy(zub_b[:, q * 512:(q + 1) * 512], bc_pts[q][:])
            lj0 = lhsT[:, 0:O_SL]
            nc.vector.tensor_tensor(lj0, nibu[:, 0:O_SL], zub_b[:], Alu.subtract)
            for q in range(2):
                nc.vector.tensor_copy(su_b[:, q * 512:(q + 1) * 512], bc_pts[2 + q][:])
            nc.vector.tensor_tensor(lj0, lj0, su_b[:], Alu.mult)
            for w in range(10):
                nc.tensor.matmul(bc_pts[w % 2][:, 0:256], warm[:, 0:128], warm[:],
                                 start=True, stop=True)
            bc_pool_ctx.__exit__(None, None, None)
            v_affine(0, 0, 1, n_act=3, glo=0, gn=4)
            v_unpack(0, 1, 3)
            for j in range(1, 4):
                u_unpack(j)
                u_dequant(j)
                v_affine(0, j, 1, n_act=2, glo=0, gn=4)
                v_unpack(0, 3 + j, 1)   # stagger second-half unpack
            v_unpack(0, 7, 1)
            for j in range(4, 8):
                u_dequant(j)
                v_affine(0, j, 1, n_act=2, glo=0, gn=4)
            # deferred: strip-0 groups 4-7 (consumed by wave B)
            for j in range(8):
                v_affine(0, j, 1, n_act=4, glo=4, gn=4)

            # ---- matmul waves ----
            def mm(pt, j, m, s, h, start, stop):
                nc.tensor.matmul(
                    pt[:],
                    lhsT[:, j * O_SL + m * 128:j * O_SL + (m + 1) * 128],
                    rhs[s][:, j * STRIP + h * 512:j * STRIP + (h + 1) * 512],
                    start=start, stop=stop)

            with tc.tile_pool(name="mps", bufs=8, space="PSUM") as mps:
                # wave A: strip 0, h 0 -- k-layered to chase dequant
                wa = [mps.tile([128, 512], F32, tag="mm", name="mmps")
                      for _ in range(8)]
                for j in range(J):
                    for m in range(8):
                        mm(wa[m], j, m, 0, 0, j == 0, j == J - 1)

                # strip-1 dequant (overlaps wave A on DVE/ACT)
                v_unpack(1, 0, 4)
                v_unpack(1, 4, 4)

                # flush wave A (ACT), dma out
                for m in range(8):
                    ot = outp.tile([128, 512], F16, tag="ot", name="ot")
                    nc.scalar.copy(ot[:], wa[m][:])
                    nc.sync.dma_start(
                        out=out[m * 128:(m + 1) * 128, 0:512], in_=ot[:])

                v_affine(1, 0, 8)

                # wave B: strip 0, h 1 -- m-grouped, inline flush
                for m in range(8):
                    tb = mps.tile([128, 512], F32, tag="mm", name="mmps")
                    for j in range(J):
                        mm(tb, j, m, 0, 1, j == 0, j == J - 1)
                    ot = outp.tile([128, 512], F16, tag="ot", name="ot")
                    nc.vector.tensor_copy(ot[:], tb[:])
                    nc.sync.dma_start(
                        out=out[m * 128:(m + 1) * 128, 512:1024], in_=ot[:])

                # waves C+D: strip 1, h 0/1 paired per m
                for m in range(8):
                    tcx = mps.tile([128, 512], F32, tag="mm", name="mmps")
                    tdx = mps.tile([128, 512], F32, tag="mm", name="mmps")
                    for j in range(J):
                        mm(tcx, j, m, 1, 0, j == 0, j == J - 1)
                        mm(tdx, j, m, 1, 1, j == 0, j == J - 1)
                    ot = outp.tile([128, 1024], F16, tag="ot2", name="ot2")
                    nc.scalar.copy(ot[:, 0:512], tcx[:])
                    nc.vector.tensor_copy(ot[:, 512:1024], tdx[:])
                    nc.sync.dma_start(
                        out=out[m * 128:(m + 1) * 128, 1024:2048], in_=ot[:])

    nc.compile()
    return nc


def _col_perm():
    """Device column i' within a 128-block vs original i: i = 8*iwl + 4h + q
    maps to i' = 32*q + 2*iwl + h. Returns idx such that O[:, orig] = dev[:, idx]."""
    i_orig = np.arange(128)
    iwl, rem = i_orig // 8, i_orig % 8
    h, q = rem // 4, rem % 4
    i_dev = 32 * q + 2 * iwl + h
    blocks = np.arange(I_SL // 128) * 128
    return (blocks[:, None] + i_dev[None, :]).reshape(-1)


def _host_prep(qweight_V, qzeros_V, scales_V, qweight_U, qzeros_U, scales_U, S):
    """Layout-only host prep: slice/permute packed int32 words + fp32 tables."""
    p = np.arange(128)
    rperm = (8 * p[:, None] + np.arange(8)[None, :]).reshape(-1)  # [p*8+j] -> r
    E128 = np.zeros((128, 128), dtype=np.float16)
    E128[np.arange(128) // 16, np.arange(128)] = 1.0
    e_i32 = np.ascontiguousarray(E128).view(np.int32)             # [128, 64]
    s_p = np.ascontiguousarray(S.reshape(128, 8))                 # S[8p+j]
    s_exp = np.ascontiguousarray(np.repeat(s_p, 16, axis=1))      # [128, 128]

    in_maps = []
    for c in range(N_CORES):
        a, b = divmod(c, P_I)
        # U words: [p, o] -> uint16 halves h-major: [p, h, o]
        qu = np.ascontiguousarray(qweight_U[:, a * O_SL:(a + 1) * O_SL])
        qu16 = qu.view(np.uint16).reshape(128, O_SL, 2)
        qut_h = np.ascontiguousarray(
            qu16.transpose(0, 2, 1).reshape(128, 2 * O_SL)).view(np.int32)

        qzu = qzeros_U[:, a * (O_SL // 8):(a + 1) * (O_SL // 8)]   # [8, 128]
        su8 = scales_U[:, a * O_SL:(a + 1) * O_SL]                 # [8, 1024]
        dma_u = np.ascontiguousarray(
            np.concatenate([qzu, su8.view(np.int32)], axis=1))

        # V words: gather r = 8p+j rows, then deinterleave uint16 halves into
        # [p, s, j, g, iwl, h] -> contiguous (g*32 + 2*iwl + h) runs
        qvT = qweight_V[b * 256:(b + 1) * 256, :].T                # [1024, 256]
        Aq = qvT[rperm].reshape(128, 8, 256)                       # [p, j, iw_g]
        v16 = Aq.view(np.uint16).reshape(128, 8, 2, 8, 16, 2)      # [p,j,s,g,iwl,h]
        qvt_h = np.ascontiguousarray(
            v16.transpose(0, 2, 1, 3, 4, 5).reshape(128, 4096)).view(np.int32)

        svs = scales_V[b * 16:(b + 1) * 16, :]                     # [16, 1024]
        svt_p = np.ascontiguousarray(svs.T[rperm].reshape(128, 128))
        qzv_t = np.ascontiguousarray(qzeros_V[b * 16:(b + 1) * 16, :].T)
        dma_v = np.ascontiguousarray(np.concatenate(
            [e_i32, qzv_t, svt_p.view(np.int32), s_p.view(np.int32),
             s_exp.view(np.int32)], axis=1))

        in_maps.append({
            "qut": qut_h, "qvt": qvt_h, "dma_u": dma_u, "dma_v": dma_v,
        })
    return in_maps


def kernel(x, qweight_V, qzeros_V, scales_V, g_idx_V,
           qweight_U, qzeros_U, scales_U, g_idx_U, S, **_unused):
    global _NC_CACHE, LAST_RESULTS
    qweight_V = np.asarray(qweight_V, dtype=np.int32)
    qzeros_V = np.asarray(qzeros_V, dtype=np.int32)
    scales_V = np.asarray(scales_V, dtype=np.float32)
    qweight_U = np.asarray(qweight_U, dtype=np.int32)
    qzeros_U = np.asarray(qzeros_U, dtype=np.int32)
    scales_U = np.asarray(scales_U, dtype=np.float32)
    S = np.asarray(S, dtype=np.float32)

    if _NC_CACHE is None:
        _NC_CACHE = _build_nc()
    nc = _NC_CACHE

    in_maps = _host_prep(qweight_V, qzeros_V, scales_V,
                         qweight_U, qzeros_U, scales_U, S)
    res = run_bass_kernel_spmd(nc, in_maps, core_ids=list(range(N_CORES)), trace=TRACE)
    LAST_RESULTS = res

    perm = _col_perm()
    O = np.empty((OUT_SIZE, IN_SIZE), dtype=np.float32)
    for c in range(N_CORES):
        a, b = divmod(c, P_I)
        O[a * O_SL:(a + 1) * O_SL, b * I_SL:(b + 1) * I_SL] = \
            res.results[c]["out"][:, perm].astype(np.float32)
    return O


# revision 21
# speedup vs baseline: 1.0294x; 1.0294x over previous
"""Trainium2 Bass kernel for nn_MixquantLinear: O = ((dequant4(V) * S) @ dequant4(U)).T.

Output O is [4096, 4096] fp32, built purely from the GPTQ-quantized weights
(the activation input `x` is dead code in the reference). Sharding: 4 slices
over output rows (o) x 2 over output cols (i) -> 8 cores, no collectives;
host concatenates the blocks.

Key ideas:
  - The contraction index r is relabeled r' = j*128 + p with r = 8p + j
    (word p, nibble j). A full-width shift+mask of the packed U words then
    directly yields k-tile j of the lhsT operand in [r, o] layout -- no PE
    transposes. V and all tables are permuted to match.
  - Packed words are viewed as uint16 halves, host-deinterleaved so every
    device unpack instruction reads AND writes contiguously (the resulting
    within-128-block i permutation is undone on the host at assembly).
  - U scale/zero tables are partition-broadcast with one K=128 indicator
    matmul; U dequant is two wide tensor_tensor ops per k-tile.
  - V dequant: per-group fused affine q*a + b (a = sv*S, b = -(zv+1)*a),
    split across ACT/DVE.
  - fp16 matmuls (k=128 tiles, N=512) accumulate fp32 in PSUM; wave A is
    k-layered to chase strip-0 dequant; output flushed as fp16.
Host-side work is layout-only (slicing/permuting packed words and tables,
column reordering and fp16->fp32 cast of the output).
"""

import numpy as np

import concourse.bass as bass
import concourse.mybir as mybir
import concourse.tile as tile
from concourse import bacc
from concourse.bass_utils import run_bass_kernel_spmd

IN_SIZE = 4096
OUT_SIZE = 4096
RANK = 1024
PACK = 8
P_O = 4
P_I = 2
O_SL = OUT_SIZE // P_O    # 1024
I_SL = IN_SIZE // P_I     # 2048
N_CORES = P_O * P_I
J = RANK // 128           # 8 k-tiles
N_STRIPS = 2
STRIP = I_SL // N_STRIPS  # 1024
NG = 16                   # V groups per core slice (I_SL / 128)

F16 = mybir.dt.float16
F32 = mybir.dt.float32
I32 = mybir.dt.int32
U16 = mybir.dt.uint16
Alu = mybir.AluOpType
Act = mybir.ActivationFunctionType

_NC_CACHE = None
TRACE = False
LAST_RESULTS = None


def _build_nc():
    nc = bacc.Bacc("TRN2", target_bir_lowering=False)

    qut = nc.dram_tensor("qut", [128, O_SL], I32, kind="ExternalInput")
    qvt = nc.dram_tensor("qvt", [128, N_STRIPS * J * 128], I32, kind="ExternalInput")
    dma_u = nc.dram_tensor("dma_u", [8, 1152], I32, kind="ExternalInput")
    dma_v = nc.dram_tensor("dma_v", [128, 344], I32, kind="ExternalInput")
    out = nc.dram_tensor("out", [O_SL, I_SL], F16, kind="ExternalOutput")

    with tile.TileContext(nc) as tc:
        with (
            tc.tile_pool(name="const", bufs=1) as cp,
            tc.tile_pool(name="outsb", bufs=4) as outp,
        ):
            qut_sb = cp.tile([128, O_SL], I32, tag="qut")
            qvt_sb = cp.tile([128, N_STRIPS * J * 128], I32, tag="qvt")
            dmu_sb = cp.tile([8, 1152], I32, tag="dmu")
            dmv_sb = cp.tile([128, 344], I32, tag="dmv")
            rhs = [cp.tile([128, J * STRIP], F16, tag=f"rhs{s}", name=f"rhs{s}")
                   for s in range(N_STRIPS)]
            nibv = [cp.tile([128, J * STRIP], U16, tag=f"nv{s}", name=f"nv{s}")
                    for s in range(N_STRIPS)]
            nibu = cp.tile([128, J * O_SL], U16, tag="nua")
            lhsT = cp.tile([128, J * O_SL], F16, tag="lhsT")
            zu8 = cp.tile([8, O_SL], I32, tag="zu8")
            rhs_bc = cp.tile([128, 2 * O_SL], F16, tag="rhsbc")
            zub_b = cp.tile([128, O_SL], F16, tag="zubb")
            su_b = cp.tile([128, O_SL], F16, tag="sub")
            zv_u = cp.tile([128, J * NG], I32, tag="zvu")
            av = cp.tile([128, J * NG], F32, tag="av")
            bvn = cp.tile([128, J * NG], F32, tag="bvn")
            tneg = cp.tile([128, J * NG], F32, tag="tneg")
            one8 = cp.tile([8, 1], F32, tag="one8")

            qzu_sb = dmu_sb[:, 0:128]
            su8 = dmu_sb[:, 128:1152].bitcast(F32)        # [8, 1024]
            e_sb = dmv_sb[:, 0:64].bitcast(F16)           # [128, 128]
            qzv_t = dmv_sb[:, 64:80]                      # [128, 16]
            svt_p = dmv_sb[:, 80:208].bitcast(F32)        # [128, 128]
            s_p = dmv_sb[:, 208:216].bitcast(F32)         # [128, 8]
            s_exp = dmv_sb[:, 216:344].bitcast(F32)       # [128, 128] S[8p+j] per col
            qvt16 = qvt_sb[:].bitcast(U16)                # [128, 8192]
            qut16 = qut_sb[:].bitcast(U16)                # [128, 2048]: h-major

            # ---- input DMAs: two HWDGE rings in parallel ----
            nc.scalar.dma_start(out=qvt_sb[:, 0:1024], in_=qvt[:, 0:1024])
            nc.sync.dma_start(out=dmv_sb[:], in_=dma_v[:])
            nc.sync.dma_start(out=dmu_sb[:], in_=dma_u[:])
            nc.sync.dma_start(out=qut_sb[:], in_=qut[:])
            nc.sync.dma_start(out=qvt_sb[:, 1024:2048], in_=qvt[:, 1024:2048])

            # ---- PE warmup: ramp the tensor-engine clock during the head ----
            warm = cp.tile([128, 256], F16, tag="warm")
            nc.vector.memset(warm[:], 0.0)
            with tc.tile_pool(name="wps", bufs=2, space="PSUM") as wps:
                wts = [wps.tile([128, 256], F32, tag="wp", name="wp")
                       for _ in range(2)]
                for w in range(44):
                    nc.tensor.matmul(wts[w % 2][:], warm[:, 0:128], warm[:],
                                     start=True, stop=True)

            # rhs_bc rows >=8 must be 0 so the indicator matmul is exact
            # (whole-tile memzero; rows 0..7 are overwritten below)
            nc.scalar.memzero(rhs_bc[:])

            # ---- U zero/scale tables on 8 partitions ----
            zu8_r = zu8[:].rearrange("p (w q) -> p w q", q=PACK)
            for jo in range(PACK):
                nc.vector.tensor_scalar(
                    out=zu8_r[:, :, jo], in0=qzu_sb, scalar1=4 * jo, scalar2=15,
                    op0=Alu.logical_shift_right, op1=Alu.bitwise_and)
            nc.vector.tensor_scalar(
                out=rhs_bc[0:8, 0:O_SL], in0=zu8[:], scalar1=1.0, scalar2=1.0,
                op0=Alu.mult, op1=Alu.add)
            nc.scalar.copy(rhs_bc[0:8, O_SL:2 * O_SL], su8)

            # ---- V tables: zeros unpack, a = sv*S, b = -(zv+1)*a ----
            for j in range(J):
                nc.vector.tensor_scalar(
                    out=zv_u[:, j * NG:(j + 1) * NG], in0=qzv_t, scalar1=4 * j,
                    scalar2=15, op0=Alu.logical_shift_right, op1=Alu.bitwise_and)
            nc.vector.tensor_tensor(av[:], svt_p, s_exp, Alu.mult)
            nc.vector.tensor_scalar(
                out=tneg[:], in0=zv_u[:], scalar1=-1.0, scalar2=-1.0,
                op0=Alu.mult, op1=Alu.add)
            nc.vector.tensor_tensor(bvn[:], tneg[:], av[:], Alu.mult)

            # ---- broadcast (zu+1)|su from 8 partitions to 128 via PE; the
            # psum->sbuf copies sit on DVE right before their consumers ----
            bc_pool_ctx = tc.tile_pool(name="bc", bufs=4, space="PSUM")
            bps = bc_pool_ctx.__enter__()
            bc_pts = []
            for q in range(4):
                pt = bps.tile([128, 512], F32, tag="bc", name="bc")
                nc.tensor.matmul(pt[:], e_sb, rhs_bc[:, q * 512:(q + 1) * 512],
                                 start=True, stop=True)
                bc_pts.append(pt)

            # ---- V unpack: contiguous uint16 shifts (host-deinterleaved) ----
            def v_unpack(s, jlo, jn):
                # in : uint16 idx = s*2048 + j*256 + g*32 + c   (c = 2*iwl + h)
                # out: nib  idx = j*1024 + g*128 + q*32 + c
                srcu = qvt16[:, s * 2048 + jlo * 256:
                             s * 2048 + (jlo + jn) * 256].rearrange(
                    "p (j g c) -> p j g c", j=jn, c=32)
                dstv = nibv[s][:, jlo * STRIP:(jlo + jn) * STRIP].rearrange(
                    "p (j g q c) -> p j g q c", j=jn, q=4, c=32)
                for q in range(4):
                    nc.vector.tensor_scalar(
                        out=dstv[:, :, :, q, :], in0=srcu, scalar1=4 * q, scalar2=15,
                        op0=Alu.logical_shift_right, op1=Alu.bitwise_and)

            # ---- U unpack: contiguous uint16 shifts -> all 8 k-tiles ----
            def u_unpack(q):
                # in : uint16 idx = h*1024 + o ; out: (4h+q)*1024 + o
                srcu = qut16[:].rearrange("p (h o) -> p h o", h=2)
                dstu = nibu[:].rearrange("p (h r) -> p h r", h=2)[
                    :, :, q * O_SL:(q + 1) * O_SL]
                nc.vector.tensor_scalar(
                    out=dstu, in0=srcu, scalar1=4 * q, scalar2=15,
                    op0=Alu.logical_shift_right, op1=Alu.bitwise_and)

            def u_dequant(j):
                lj = lhsT[:, j * O_SL:(j + 1) * O_SL]
                nc.vector.tensor_tensor(
                    lj, nibu[:, j * O_SL:(j + 1) * O_SL], zub_b[:], Alu.subtract)
                nc.vector.tensor_tensor(lj, lj, su_b[:], Alu.mult)

            def v_affine(s, jlo, jn, n_act=4, glo=0, gn=8):
                for j in range(jlo, jlo + jn):
                    for g in range(glo, glo + gn):
                        col = j * NG + s * 8 + g
                        o_ap = rhs[s][:, j * STRIP + g * 128:j * STRIP + (g + 1) * 128]
                        i_ap = nibv[s][:, j * STRIP + g * 128:j * STRIP + (g + 1) * 128]
                        if g < n_act:
                            nc.scalar.activation(
                                o_ap, i_ap, Act.Identity,
                                bias=bvn[:, col:col + 1], scale=av[:, col:col + 1])
                        else:
                            nc.vector.tensor_scalar(
                                out=o_ap, in0=i_ap, scalar1=av[:, col:col + 1],
                                scalar2=bvn[:, col:col + 1], op0=Alu.mult, op1=Alu.add)

            # strip-0: per-j chase (u_unpack(q) yields tiles q and q+4)
            v_unpack(0, 0, 1)
            u_unpack(0)
            for q in range(2):
                nc.vector.tensor_copy(zub_b[:, q * 512:(q + 1) * 512], bc_pts[q][:])
            lj0 = lhsT[:, 0:O_SL]
            nc.vector.tensor_tensor(lj0, nibu[:, 0:O_SL], zub_b[:], Alu.subtract)
            for q in range(2):
                nc.vector.tensor_copy(su_b[:, q * 512:(q + 1) * 512], bc_pts[2 + q][:])
            nc.vector.tensor_tensor(lj0, lj0, su_b[:], Alu.mult)
            for w in range(10):
                nc.tensor.matmul(bc_pts[w % 2][:, 0:256], warm[:, 0:128], warm[:],
                                 start=True, stop=True)
            bc_pool_ctx.__exit__(None, None, None)
            v_affine(0, 0, 1, n_act=6)
            v_unpack(0, 1, 3)
            for j in range(1, 4):
                u_unpack(j)
                u_dequant(j)
                v_affine(0, j, 1, n_act=4)
                v_unpack(0, 3 + j, 1)   # stagger second-half unpack
            v_unpack(0, 7, 1)
            for j in range(4, 8):
                u_dequant(j)
                v_affine(0, j, 1, n_act=3)

            # ---- matmul waves ----
            def mm(pt, j, m, s, h, start, stop):
                nc.tensor.matmul(
                    pt[:],
                    lhsT[:, j * O_SL + m * 128:j * O_SL + (m + 1) * 128],
                    rhs[s][:, j * STRIP + h * 512:j * STRIP + (h + 1) * 512],
                    start=start, stop=stop)

            with tc.tile_pool(name="mps", bufs=8, space="PSUM") as mps:
                # wave A: strip 0, h 0 -- k-layered to chase dequant
                wa = [mps.tile([128, 512], F32, tag="mm", name="mmps")
                      for _ in range(8)]
                for j in range(J):
                    for m in range(8):
                        mm(wa[m], j, m, 0, 0, j == 0, j == J - 1)

                # strip-1 dequant (overlaps wave A on DVE/ACT)
                v_unpack(1, 0, 4)
                v_unpack(1, 4, 4)

                # flush wave A (ACT), dma out
                for m in range(8):
                    ot = outp.tile([128, 512], F16, tag="ot", name="ot")
                    nc.scalar.copy(ot[:], wa[m][:])
                    nc.sync.dma_start(
                        out=out[m * 128:(m + 1) * 128, 0:512], in_=ot[:])

                v_affine(1, 0, 8)

                # wave B: strip 0, h 1 -- m-grouped, inline flush
                for m in range(8):
                    tb = mps.tile([128, 512], F32, tag="mm", name="mmps")
                    for j in range(J):
                        mm(tb, j, m, 0, 1, j == 0, j == J - 1)
                    ot = outp.tile([128, 512], F16, tag="ot", name="ot")
                    nc.vector.tensor_copy(ot[:], tb[:])
                    nc.sync.dma_start(
                        out=out[m * 128:(m + 1) * 128, 512:1024], in_=ot[:])

                # waves C+D: strip 1, h 0/1 paired per m
                for m in range(8):
                    tcx = mps.tile([128, 512], F32, tag="mm", name="mmps")
                    tdx = mps.tile([128, 512], F32, tag="mm", name="mmps")
                    for j in range(J):
                        mm(tcx, j, m, 1, 0, j == 0, j == J - 1)
                        mm(tdx, j, m, 1, 1, j == 0, j == J - 1)
                    ot = outp.tile([128, 1024], F16, tag="ot2", name="ot2")
                    nc.scalar.copy(ot[:, 0:512], tcx[:])
                    nc.vector.tensor_copy(ot[:, 512:1024], tdx[:])
                    nc.sync.dma_start(
                        out=out[m * 128:(m + 1) * 128, 1024:2048], in_=ot[:])

    nc.compile()
    return nc


def _col_perm():
    """Device column i' within a 128-block vs original i: i = 8*iwl + 4h + q
    maps to i' = 32*q + 2*iwl + h. Returns idx such that O[:, orig] = dev[:, idx]."""
    i_orig = np.arange(128)
    iwl, rem = i_orig // 8, i_orig % 8
    h, q = rem // 4, rem % 4
    i_dev = 32 * q + 2 * iwl + h
    blocks = np.arange(I_SL // 128) * 128
    return (blocks[:, None] + i_dev[None, :]).reshape(-1)


def _host_prep(qweight_V, qzeros_V, scales_V, qweight_U, qzeros_U, scales_U, S):
    """Layout-only host prep: slice/permute packed int32 words + fp32 tables."""
    p = np.arange(128)
    rperm = (8 * p[:, None] + np.arange(8)[None, :]).reshape(-1)  # [p*8+j] -> r
    E128 = np.zeros((128, 128), dtype=np.float16)
    E128[np.arange(128) // 16, np.arange(128)] = 1.0
    e_i32 = np.ascontiguousarray(E128).view(np.int32)             # [128, 64]
    s_p = np.ascontiguousarray(S.reshape(128, 8))                 # S[8p+j]
    s_exp = np.ascontiguousarray(np.repeat(s_p, 16, axis=1))      # [128, 128]

    in_maps = []
    for c in range(N_CORES):
        a, b = divmod(c, P_I)
        # U words: [p, o] -> uint16 halves h-major: [p, h, o]
        qu = np.ascontiguousarray(qweight_U[:, a * O_SL:(a + 1) * O_SL])
        qu16 = qu.view(np.uint16).reshape(128, O_SL, 2)
        qut_h = np.ascontiguousarray(
            qu16.transpose(0, 2, 1).reshape(128, 2 * O_SL)).view(np.int32)

        qzu = qzeros_U[:, a * (O_SL // 8):(a + 1) * (O_SL // 8)]   # [8, 128]
        su8 = scales_U[:, a * O_SL:(a + 1) * O_SL]                 # [8, 1024]
        dma_u = np.ascontiguousarray(
            np.concatenate([qzu, su8.view(np.int32)], axis=1))

        # V words: gather r = 8p+j rows, then deinterleave uint16 halves into
        # [p, s, j, g, iwl, h] -> contiguous (g*32 + 2*iwl + h) runs
        qvT = qweight_V[b * 256:(b + 1) * 256, :].T                # [1024, 256]
        Aq = qvT[rperm].reshape(128, 8, 256)                       # [p, j, iw_g]
        v16 = Aq.view(np.uint16).reshape(128, 8, 2, 8, 16, 2)      # [p,j,s,g,iwl,h]
        qvt_h = np.ascontiguousarray(
            v16.transpose(0, 2, 1, 3, 4, 5).reshape(128, 4096)).view(np.int32)

        svs = scales_V[b * 16:(b + 1) * 16, :]                     # [16, 1024]
        svt_p = np.ascontiguousarray(svs.T[rperm].reshape(128, 128))
        qzv_t = np.ascontiguousarray(qzeros_V[b * 16:(b + 1) * 16, :].T)
        dma_v = np.ascontiguousarray(np.concatenate(
            [e_i32, qzv_t, svt_p.view(np.int32), s_p.view(np.int32),
             s_exp.view(np.int32)], axis=1))

        in_maps.append({
            "qut": qut_h, "qvt": qvt_h, "dma_u": dma_u, "dma_v": dma_v,
        })
    return in_maps


def kernel(x, qweight_V, qzeros_V, scales_V, g_idx_V,
           qweight_U, qzeros_U, scales_U, g_idx_U, S, **_unused):
    global _NC_CACHE, LAST_RESULTS
    qweight_V = np.asarray(qweight_V, dtype=np.int32)
    qzeros_V = np.asarray(qzeros_V, dtype=np.int32)
    scales_V = np.asarray(scales_V, dtype=np.float32)
    qweight_U = np.asarray(qweight_U, dtype=np.int32)
    qzeros_U = np.asarray(qzeros_U, dtype=np.int32)
    scales_U = np.asarray(scales_U, dtype=np.float32)
    S = np.asarray(S, dtype=np.float32)

    if _NC_CACHE is None:
        _NC_CACHE = _build_nc()
    nc = _NC_CACHE

    in_maps = _host_prep(qweight_V, qzeros_V, scales_V,
                         qweight_U, qzeros_U, scales_U, S)
    res = run_bass_kernel_spmd(nc, in_maps, core_ids=list(range(N_CORES)), trace=TRACE)
    LAST_RESULTS = res

    perm = _col_perm()
    O = np.empty((OUT_SIZE, IN_SIZE), dtype=np.float32)
    for c in range(N_CORES):
        a, b = divmod(c, P_I)
        O[a * O_SL:(a + 1) * O_SL, b * I_SL:(b + 1) * I_SL] = \
            res.results[c]["out"][:, perm].astype(np.float32)
    return O
